# revision 6
# baseline (speedup 1.0000x reference)
"""Expert-parallel MoE FFN kernel for Trainium2 (8 NeuronCores), v2.

Problem: inputs [B=2, E=8, C=8192, H=512], per-expert FFN
    h   = gelu_tanh(x_e @ w1_e + b1_e)        (w1: [E, H, F=2048])
    out = h @ w2_e + b2_e                     (w2: [E, F, H])

Sharding: expert-parallel - core e owns expert e's tokens [B*C, H] and
weights; no cross-core communication.

v2 changes vs v1 (each validated by HW-loop slope timing):
  - x is transposed on the HOST (input marshaling inside kernel()), so
    the device kernel has ZERO PE transposes (v1 spent ~275ns per
    128x128 transpose-mode op).
  - bf16 matmuls (host casts x/w1/w2).  Measured: bf16 streams at the
    same 1 col/cycle as f32r on TRN2 (N=512 MM ~213-220ns), so this
    buys DMA/SBUF headroom, not PE time.  rel_err ~3.4e-3 (gate 2e-2).
  - GEMM2(m) and GEMM1(m+1) interleaved at chain granularity (1 G2
    chain then 4 G1 chains per token-block) so gelu evictions never
    gate the PE; PSUM pools 5+3 banks; deep SBUF pools (4,3,4).
  - Measured budget per core: 4096 MMs x ~213ns stream = 873us floor;
    +~130us intrinsic accumulation-chain restart cost (G1 chains are
    only 4 MMs: K=H=512), +~50us eviction coupling -> ~1.05-1.08ms.

Per-core dataflow:
    GEMM1: ph[f-chunk, tok] += w1[hk, f-chunk].T @ xT[hk, tok-tile]
    gelu on ACT during PSUM->SBUF eviction (bias = b1 per-partition);
    odd groups go via DVE copy + ACT-from-SBUF to avoid back-to-back
    ACT PSUM reads (collapses PE throughput; measured in v1).
    GEMM2: out[tok-blk, :] += hT[fk, tok-blk].T @ w2[fk, :]
    b2 add on DVE during PSUM->SBUF eviction.
"""

import contextlib

import numpy as np

_B, _E, _C, _H, _F = 2, 8, 8192, 512, 2048
_TOK = _B * _C  # 16384 tokens per expert
_P = 128
_T = 512  # tokens per macro tile

_MM_MODE = "bf16"  # "bf16" | "f32r"

_cache = {}


def build_nc(tok=_TOK, act_name="Gelu_apprx_tanh", n_devices=_E,
             mm_mode=_MM_MODE, loop_reps=1, skip=(), psum_cfg=(5, 3),
             g1_per_g2=4, evict_mode="alt", T=_T, bufs_cfg=(4, 3, 4),
             g2_first=True):
    import concourse.mybir as mybir
    import concourse.tile as tile
    from concourse import bacc

    H, F, P = _H, _F, _P
    HK, FM = H // P, F // P  # 4 contraction chunks, 16 F groups
    FK = FM
    wide = evict_mode == "wide"  # paired G1 chains + single wide gelu
    if wide:
        assert T == 1024
    NJ = T // P  # token sub-blocks per macro tile
    NM = tok // T  # macro tiles
    g1_per_g2 = FM // NJ if not wide else None
    f32 = mybir.dt.float32
    mmdt = mybir.dt.bfloat16 if mm_mode == "bf16" else mybir.dt.float32r
    act = getattr(mybir.ActivationFunctionType, act_name)

    nc = bacc.Bacc("TRN2", debug=False, target_bir_lowering=False,
                   num_devices=n_devices)
    xT = nc.dram_tensor("xT", [H, tok], mmdt, kind="ExternalInput").ap()
    w1 = nc.dram_tensor("w1", [H, F], mmdt, kind="ExternalInput").ap()
    b1 = nc.dram_tensor("b1", [F], f32, kind="ExternalInput").ap()
    w2 = nc.dram_tensor("w2", [F, H], mmdt, kind="ExternalInput").ap()
    b2 = nc.dram_tensor("b2", [H], f32, kind="ExternalInput").ap()
    out = nc.dram_tensor("out", [tok, H], f32, kind="ExternalOutput").ap()

    with tile.TileContext(nc) as tc:
        with (
            tc.tile_pool(name="const", bufs=1) as const,
            tc.tile_pool(name="xin", bufs=bufs_cfg[0]) as xin_pool,
            tc.tile_pool(name="ht", bufs=bufs_cfg[1]) as ht_pool,
            tc.tile_pool(name="obuf", bufs=2) as o_pool,
            tc.tile_pool(name="scr", bufs=bufs_cfg[2]) as scratch,
            tc.tile_pool(name="ps1", bufs=psum_cfg[0], space="PSUM") as ps1,
            tc.tile_pool(name="ps2", bufs=psum_cfg[1], space="PSUM") as ps2,
        ):
            # --- weights / constants, resident in SBUF for the whole kernel
            w1_sb = const.tile([P, HK, F], mmdt)
            w2_sb = const.tile([P, FK, H], mmdt)
            nc.sync.dma_start(
                w1_sb[:], w1.rearrange("(hk p) f -> p hk f", p=P))
            nc.sync.dma_start(
                w2_sb[:], w2.rearrange("(fk p) h -> p fk h", p=P))
            b1_sb = const.tile([P, FM], f32)
            nc.sync.dma_start(b1_sb[:], b1.rearrange("(fm p) -> p fm", p=P))
            b2_row = const.tile([1, H], f32)
            nc.sync.dma_start(b2_row[:], b2[None, :])
            ones = const.tile([1, P], f32)
            nc.any.memset(ones[:], 1.0)
            # broadcast b2 across all 128 partitions via a K=1 matmul
            b2_bc = const.tile([P, H], f32)
            ps_b2 = ps2.tile([P, H], f32, tag="po")
            nc.tensor.matmul(ps_b2[:], ones[:], b2_row[:], start=True,
                             stop=True)
            nc.vector.tensor_copy(b2_bc[:], ps_b2[:])
            static_ht = None
            if "evict" in skip or "gemm1" in skip:  # timing-only variants
                static_ht = const.tile([P, FM, T], mmdt, name="static_ht")
                nc.any.memset(static_ht.rearrange("p fm t -> p (fm t)"), 0.0)

            def load_x(m):
                xk = xin_pool.tile([P, HK, T], mmdt, name=f"xk{m}", tag="xk")
                nc.sync.dma_start(
                    xk[:],
                    xT[:, m * T:(m + 1) * T].rearrange(
                        "(hk p) t -> p hk t", p=P))
                return xk

            def g1_group(xk, ht, m, fm):
                """One GEMM1 accumulation group (4 matmuls) + gelu evict.

                Evictions alternate engines: even fm goes ACT gelu
                straight from PSUM; odd fm is DVE-copied to SBUF scratch
                first and gelu'd from there (back-to-back ACT PSUM reads
                collapse PE throughput; measured in v1).
                """
                ph = ps1.tile([P, T], f32, name=f"ph{m}_{fm}", tag="ph")
                for hk in range(HK):
                    nc.tensor.matmul(
                        ph[:],
                        w1_sb[:, hk, fm * P:(fm + 1) * P],
                        xk[:, hk, :],
                        start=(hk == 0),
                        stop=(hk == HK - 1),
                    )
                if "evict" in skip:
                    return
                direct = (evict_mode == "act" or
                          (evict_mode == "alt" and fm % 2 == 0))
                if direct:
                    nc.scalar.activation(
                        ht[:, fm, :], ph[:], act, bias=b1_sb[:, fm:fm + 1])
                else:
                    sc = scratch.tile([P, T], f32, name=f"sc{m}_{fm}",
                                      tag="sc")
                    nc.vector.tensor_copy(sc[:], ph[:])
                    nc.scalar.activation(
                        ht[:, fm, :], sc[:], act, bias=b1_sb[:, fm:fm + 1])

            def g1_pair(xk, ht, m, fm):
                """Two GEMM1 chains for the SAME fm over adjacent 512-token
                halves -> one 2-bank PSUM tile, evicted by a single wide
                [128,1024] ACT gelu.  Both halves share fm's per-partition
                bias, so this is exact for any b1."""
                ph = ps1.tile([P, 2, T // 2], f32, name=f"php{m}_{fm}",
                              tag="ph")
                for half in range(2):
                    for hk in range(HK):
                        nc.tensor.matmul(
                            ph[:, half, :],
                            w1_sb[:, hk, fm * P:(fm + 1) * P],
                            xk[:, hk, half * (T // 2):(half + 1) * (T // 2)],
                            start=(hk == 0),
                            stop=(hk == HK - 1),
                        )
                if "evict" in skip:
                    return
                nc.scalar.activation(
                    ht[:, fm, :], ph.rearrange("p h t -> p (h t)"), act,
                    bias=b1_sb[:, fm:fm + 1])

            def g2_group(ht, obig, j):
                """One GEMM2 accumulation group (16 matmuls) + b2 evict."""
                po = ps2.tile([P, H], f32, tag="po", name="po")
                for fk in range(FK):
                    nc.tensor.matmul(
                        po[:],
                        ht[:, fk, j * P:(j + 1) * P],
                        w2_sb[:, fk, :],
                        start=(fk == 0),
                        stop=(fk == FK - 1),
                    )
                if "evict" not in skip:
                    nc.vector.tensor_add(obig[:, j, :], po[:], b2_bc[:])

            def new_ht(m):
                if static_ht is not None:
                    return static_ht
                return ht_pool.tile([P, FM, T], mmdt, name=f"ht{m}", tag="ht")

            do_g1 = "gemm1" not in skip
            do_g2 = "gemm2" not in skip
            # loop_reps > 1 wraps the body in a hardware loop - used only
            # by the timing harness to amortize per-dispatch overhead.
            reps_ctx = (tc.For_i(0, loop_reps, 1) if loop_reps > 1
                        else contextlib.nullcontext())
            with reps_ctx:
                # software pipeline, one macro of skew: G1 fills ht(m+1)
                # while G2 consumes ht(m).
                def g1_units(xk, ht, m, j):
                    """G1 emission units for token-slot j of macro m."""
                    if wide:  # 2 pairs per j-slot (16 pairs over 8 slots)
                        for q in range(2):
                            g1_pair(xk, ht, m, 2 * j + q)
                    else:
                        for q in range(g1_per_g2):
                            g1_group(xk, ht, m, g1_per_g2 * j + q)

                xk = load_x(0)
                xk_next = load_x(1)
                ht = new_ht(0)
                if do_g1:
                    for j in range(NJ):
                        g1_units(xk, ht, 0, j)
                for m in range(NM):
                    if m + 2 < NM:
                        xk_after = load_x(m + 2)
                    else:
                        xk_after = None
                    ht_next = new_ht(m + 1) if (m + 1 < NM) else None
                    obig = (None if "evict" in skip else
                            o_pool.tile([P, NJ, H], f32, name=f"ob{m}",
                                        tag="ob"))
                    for j in range(NJ):
                        if g2_first and do_g2:
                            g2_group(ht, obig, j)
                        if ht_next is not None and do_g1:
                            g1_units(xk_next, ht_next, m + 1, j)
                        if do_g2 and not g2_first:
                            g2_group(ht, obig, j)
                    if do_g2 and "outdma" not in skip and "evict" not in skip:
                        nc.sync.dma_start(
                            out[m * T:(m + 1) * T, :].rearrange(
                                "(j p) h -> p j h", p=P),
                            obig[:])
                    xk, xk_next, ht = xk_next, xk_after, ht_next

    nc.compile()
    return nc


def make_in_maps(inputs_np, mm_mode=_MM_MODE):
    import ml_dtypes

    inputs = np.asarray(inputs_np["inputs"], dtype=np.float32)
    w1 = np.asarray(inputs_np["w1"], dtype=np.float32)
    b1 = np.asarray(inputs_np["b1"], dtype=np.float32)
    w2 = np.asarray(inputs_np["w2"], dtype=np.float32)
    b2 = np.asarray(inputs_np["b2"], dtype=np.float32)

    B, E, C, H = inputs.shape
    tok = B * C
    # [B, E, C, H] -> per-expert TRANSPOSED token matrix [E, H, B*C]
    xT = np.ascontiguousarray(
        inputs.transpose(1, 3, 0, 2).reshape(E, H, tok))
    if mm_mode == "bf16":
        cast = lambda a: np.ascontiguousarray(a).astype(ml_dtypes.bfloat16)
    else:
        cast = lambda a: np.ascontiguousarray(a)

    return [
        {
            "xT": cast(xT[e]),
            "w1": cast(w1[e]),
            "b1": np.ascontiguousarray(b1[e]),
            "w2": cast(w2[e]),
            "b2": np.ascontiguousarray(b2[e]),
        }
        for e in range(E)
    ]


def kernel(inputs, w1, b1, w2, b2):
    from concourse.bass_utils import run_bass_kernel_spmd

    B, E, C, H = inputs.shape

    if "nc" not in _cache:
        _cache["nc"] = build_nc()
    nc = _cache["nc"]

    in_maps = make_in_maps(
        {"inputs": inputs, "w1": w1, "b1": b1, "w2": w2, "b2": b2})
    res = run_bass_kernel_spmd(nc, in_maps, core_ids=list(range(E)))
    o = np.stack([res.results[e]["out"] for e in range(E)])  # [E, tok, H]
    return np.ascontiguousarray(
        o.reshape(E, B, C, H).transpose(1, 0, 2, 3))


# revision 12
# speedup vs baseline: 1.0618x; 1.0618x over previous
"""Expert-parallel MoE FFN kernel for Trainium2 (8 NeuronCores), v2.

Problem: inputs [B=2, E=8, C=8192, H=512], per-expert FFN
    h   = gelu_tanh(x_e @ w1_e + b1_e)        (w1: [E, H, F=2048])
    out = h @ w2_e + b2_e                     (w2: [E, F, H])

Sharding: expert-parallel - core e owns expert e's tokens [B*C, H] and
weights; no cross-core communication.

v2 changes vs v1 (each validated by HW-loop slope timing):
  - x is transposed on the HOST (input marshaling inside kernel()), so
    the device kernel has ZERO PE transposes (v1 spent ~275ns per
    128x128 transpose-mode op).
  - bf16 matmuls (host casts x/w1/w2).  Measured: bf16 streams at the
    same 1 col/cycle as f32r on TRN2 (N=512 MM ~213-220ns), so this
    buys DMA/SBUF headroom, not PE time.  rel_err ~3.4e-3 (gate 2e-2).
  - GEMM2(m) and GEMM1(m+1) interleaved at chain granularity (1 G2
    chain then 4 G1 chains per token-block) so gelu evictions never
    gate the PE; PSUM pools 5+3 banks; deep SBUF pools (4,3,4).
  - Measured budget per core: 4096 MMs x ~213ns stream = 873us floor;
    +~130us intrinsic accumulation-chain restart cost (G1 chains are
    only 4 MMs: K=H=512), +~50us eviction coupling -> ~1.05-1.08ms.

Per-core dataflow:
    GEMM1: ph[f-chunk, tok] += w1[hk, f-chunk].T @ xT[hk, tok-tile]
    gelu on ACT during PSUM->SBUF eviction (bias = b1 per-partition);
    odd groups go via DVE copy + ACT-from-SBUF to avoid back-to-back
    ACT PSUM reads (collapses PE throughput; measured in v1).
    GEMM2: out[tok-blk, :] += hT[fk, tok-blk].T @ w2[fk, :]
    b2 add on DVE during PSUM->SBUF eviction.
"""

import contextlib

import numpy as np

_B, _E, _C, _H, _F = 2, 8, 8192, 512, 2048
_TOK = _B * _C  # 16384 tokens per expert
_P = 128
_T = 512  # tokens per macro tile

_MM_MODE = "bf16"  # "bf16" | "f32r"

_cache = {}


def build_nc(tok=_TOK, act_name="Gelu_apprx_tanh", n_devices=_E,
             mm_mode=_MM_MODE, loop_reps=1, skip=(), psum_cfg=(5, 3),
             g1_per_g2=4, evict_mode="alt", T=_T, bufs_cfg=(4, 3, 4),
             g2_first=True):
    import concourse.mybir as mybir
    import concourse.tile as tile
    from concourse import bacc

    H, F, P = _H, _F, _P
    HK, FM = H // P, F // P  # 4 contraction chunks, 16 F groups
    FK = FM
    wide = evict_mode == "wide"  # paired G1 chains + single wide gelu
    if wide:
        assert T == 1024
    NJ = T // P  # token sub-blocks per macro tile
    NM = tok // T  # macro tiles
    g1_per_g2 = FM // NJ if not wide else None
    f32 = mybir.dt.float32
    mmdt = mybir.dt.bfloat16 if mm_mode == "bf16" else mybir.dt.float32r
    act = getattr(mybir.ActivationFunctionType, act_name)

    nc = bacc.Bacc("TRN2", debug=False, target_bir_lowering=False,
                   num_devices=n_devices)
    xT = nc.dram_tensor("xT", [H, tok], mmdt, kind="ExternalInput").ap()
    w1 = nc.dram_tensor("w1", [H, F], mmdt, kind="ExternalInput").ap()
    b1 = nc.dram_tensor("b1", [F], f32, kind="ExternalInput").ap()
    w2 = nc.dram_tensor("w2", [F, H], mmdt, kind="ExternalInput").ap()
    b2 = nc.dram_tensor("b2", [H], f32, kind="ExternalInput").ap()
    out = nc.dram_tensor("out", [tok, H], f32, kind="ExternalOutput").ap()

    with tile.TileContext(nc) as tc:
        with (
            tc.tile_pool(name="const", bufs=1) as const,
            tc.tile_pool(name="xin", bufs=bufs_cfg[0]) as xin_pool,
            tc.tile_pool(name="ht", bufs=bufs_cfg[1]) as ht_pool,
            tc.tile_pool(name="obuf", bufs=2) as o_pool,
            tc.tile_pool(name="scr", bufs=bufs_cfg[2]) as scratch,
            tc.tile_pool(name="ps1", bufs=psum_cfg[0], space="PSUM") as ps1,
            tc.tile_pool(name="ps2", bufs=psum_cfg[1], space="PSUM") as ps2,
        ):
            # --- weights / constants, resident in SBUF for the whole kernel
            # weights DMA'd in quarters so the first G1 chains only wait
            # for the first 1MB, overlapping the rest with compute (only
            # matters for the single-dispatch prologue, not the For_i
            # steady state)
            w1_sb = const.tile([P, HK, F], mmdt)
            w2_sb = const.tile([P, FK, H], mmdt)
            for q in range(4):
                fs = q * (F // 4)
                nc.sync.dma_start(
                    w1_sb[:, :, fs:fs + F // 4],
                    w1[:, fs:fs + F // 4].rearrange(
                        "(hk p) f -> p hk f", p=P))
            for q in range(4):
                ks = q * (FK // 4)
                nc.sync.dma_start(
                    w2_sb[:, ks:ks + FK // 4, :],
                    w2[ks * P:(ks + FK // 4) * P, :].rearrange(
                        "(fk p) h -> p fk h", p=P))
            b1_sb = const.tile([P, FM], f32)
            nc.sync.dma_start(b1_sb[:], b1.rearrange("(fm p) -> p fm", p=P))
            b2_row = const.tile([1, H], f32)
            nc.sync.dma_start(b2_row[:], b2[None, :])
            ones = const.tile([1, P], f32)
            nc.any.memset(ones[:], 1.0)
            # broadcast b2 across all 128 partitions via a K=1 matmul
            b2_bc = const.tile([P, H], f32)
            ps_b2 = ps2.tile([P, H], f32, tag="po")
            nc.tensor.matmul(ps_b2[:], ones[:], b2_row[:], start=True,
                             stop=True)
            nc.vector.tensor_copy(b2_bc[:], ps_b2[:])
            static_ht = None
            if "evict" in skip or "gemm1" in skip:  # timing-only variants
                static_ht = const.tile([P, FM, T], mmdt, name="static_ht")
                nc.any.memset(static_ht.rearrange("p fm t -> p (fm t)"), 0.0)

            def load_x(m):
                xk = xin_pool.tile([P, HK, T], mmdt, name=f"xk{m}", tag="xk")
                nc.sync.dma_start(
                    xk[:],
                    xT[:, m * T:(m + 1) * T].rearrange(
                        "(hk p) t -> p hk t", p=P))
                return xk

            def g1_group(xk, ht, m, fm):
                """One GEMM1 accumulation group (4 matmuls) + gelu evict.

                Evictions alternate engines: even fm goes ACT gelu
                straight from PSUM; odd fm is DVE-copied to SBUF scratch
                first and gelu'd from there (back-to-back ACT PSUM reads
                collapse PE throughput; measured in v1).
                """
                ph = ps1.tile([P, T], f32, name=f"ph{m}_{fm}", tag="ph")
                for hk in range(HK):
                    nc.tensor.matmul(
                        ph[:],
                        w1_sb[:, hk, fm * P:(fm + 1) * P],
                        xk[:, hk, :],
                        start=(hk == 0),
                        stop=(hk == HK - 1),
                    )
                if "evict" in skip:
                    return
                direct = (evict_mode == "act" or
                          (evict_mode == "alt" and fm % 2 == 0))
                if direct:
                    nc.scalar.activation(
                        ht[:, fm, :], ph[:], act, bias=b1_sb[:, fm:fm + 1])
                else:
                    sc = scratch.tile([P, T], f32, name=f"sc{m}_{fm}",
                                      tag="sc")
                    nc.vector.tensor_copy(sc[:], ph[:])
                    nc.scalar.activation(
                        ht[:, fm, :], sc[:], act, bias=b1_sb[:, fm:fm + 1])

            def g1_duo(xk, ht, m, fm):
                """Two G1 chains (fm, fm+1) with their matmuls interleaved
                A0 B0 A1 B1 ... across two PSUM banks.  Accumulation state
                is per-bank (has_written), so two open groups may legally
                interleave; if the chain-restart cost is a start-after-stop
                pipeline break, the sibling chain's streaming hides it."""
                pha = ps1.tile([P, T], f32, name=f"ph{m}_{fm}", tag="ph")
                phb = ps1.tile([P, T], f32, name=f"ph{m}_{fm + 1}", tag="ph")
                for hk in range(HK):
                    for ph, f in ((pha, fm), (phb, fm + 1)):
                        nc.tensor.matmul(
                            ph[:],
                            w1_sb[:, hk, f * P:(f + 1) * P],
                            xk[:, hk, :],
                            start=(hk == 0),
                            stop=(hk == HK - 1),
                            skip_group_check=True,
                        )
                if "evict" in skip:
                    return
                nc.scalar.activation(
                    ht[:, fm, :], pha[:], act, bias=b1_sb[:, fm:fm + 1])
                sc = scratch.tile([P, T], f32, name=f"sc{m}_{fm}", tag="sc")
                nc.vector.tensor_copy(sc[:], phb[:])
                nc.scalar.activation(
                    ht[:, fm + 1, :], sc[:], act,
                    bias=b1_sb[:, fm + 1:fm + 2])

            def g1_quad(xk, ht, m, fm):
                """Four G1 chains (fm..fm+3) interleaved at the matmul
                level across four PSUM banks."""
                phs = [ps1.tile([P, T], f32, name=f"ph{m}_{fm + q}",
                                tag="ph") for q in range(4)]
                for hk in range(HK):
                    for q in range(4):
                        nc.tensor.matmul(
                            phs[q][:],
                            w1_sb[:, hk, (fm + q) * P:(fm + q + 1) * P],
                            xk[:, hk, :],
                            start=(hk == 0),
                            stop=(hk == HK - 1),
                            skip_group_check=True,
                        )
                if "evict" in skip:
                    return
                for q in range(4):
                    f = fm + q
                    if q % 2 == 0:
                        nc.scalar.activation(
                            ht[:, f, :], phs[q][:], act,
                            bias=b1_sb[:, f:f + 1])
                    else:
                        sc = scratch.tile([P, T], f32, name=f"sc{m}_{f}",
                                          tag="sc")
                        nc.vector.tensor_copy(sc[:], phs[q][:])
                        nc.scalar.activation(
                            ht[:, f, :], sc[:], act, bias=b1_sb[:, f:f + 1])

            def g1_pair(xk, ht, m, fm):
                """Two GEMM1 chains for the SAME fm over adjacent 512-token
                halves -> one 2-bank PSUM tile, evicted by a single wide
                [128,1024] ACT gelu.  Both halves share fm's per-partition
                bias, so this is exact for any b1."""
                ph = ps1.tile([P, 2, T // 2], f32, name=f"php{m}_{fm}",
                              tag="ph")
                for half in range(2):
                    for hk in range(HK):
                        nc.tensor.matmul(
                            ph[:, half, :],
                            w1_sb[:, hk, fm * P:(fm + 1) * P],
                            xk[:, hk, half * (T // 2):(half + 1) * (T // 2)],
                            start=(hk == 0),
                            stop=(hk == HK - 1),
                        )
                if "evict" in skip:
                    return
                nc.scalar.activation(
                    ht[:, fm, :], ph.rearrange("p h t -> p (h t)"), act,
                    bias=b1_sb[:, fm:fm + 1])

            def g2_group(ht, obig, j):
                """One GEMM2 accumulation group (16 matmuls) + b2 evict."""
                po = ps2.tile([P, H], f32, tag="po", name="po")
                for fk in range(FK):
                    nc.tensor.matmul(
                        po[:],
                        ht[:, fk, j * P:(j + 1) * P],
                        w2_sb[:, fk, :],
                        start=(fk == 0),
                        stop=(fk == FK - 1),
                    )
                if "evict" not in skip:
                    nc.vector.tensor_add(obig[:, j, :], po[:], b2_bc[:])

            def new_ht(m):
                if static_ht is not None:
                    return static_ht
                return ht_pool.tile([P, FM, T], mmdt, name=f"ht{m}", tag="ht")

            do_g1 = "gemm1" not in skip
            do_g2 = "gemm2" not in skip
            # loop_reps > 1 wraps the body in a hardware loop - used only
            # by the timing harness to amortize per-dispatch overhead.
            reps_ctx = (tc.For_i(0, loop_reps, 1) if loop_reps > 1
                        else contextlib.nullcontext())
            with reps_ctx:
                # software pipeline, one macro of skew: G1 fills ht(m+1)
                # while G2 consumes ht(m).
                def g1_units(xk, ht, m, j):
                    """G1 emission units for token-slot j of macro m."""
                    if wide:  # 2 pairs per j-slot (16 pairs over 8 slots)
                        for q in range(2):
                            g1_pair(xk, ht, m, 2 * j + q)
                    elif evict_mode == "duo":
                        for q in range(g1_per_g2 // 2):
                            g1_duo(xk, ht, m, g1_per_g2 * j + 2 * q)
                    elif evict_mode == "quad":
                        g1_quad(xk, ht, m, g1_per_g2 * j)
                    else:
                        for q in range(g1_per_g2):
                            g1_group(xk, ht, m, g1_per_g2 * j + q)

                xk = load_x(0)
                xk_next = load_x(1)
                ht = new_ht(0)
                if do_g1:
                    for j in range(NJ):
                        g1_units(xk, ht, 0, j)
                for m in range(NM):
                    if m + 2 < NM:
                        xk_after = load_x(m + 2)
                    else:
                        xk_after = None
                    ht_next = new_ht(m + 1) if (m + 1 < NM) else None
                    obig = (None if "evict" in skip else
                            o_pool.tile([P, NJ, H], f32, name=f"ob{m}",
                                        tag="ob"))
                    for j in range(NJ):
                        if g2_first == "mid" and not wide:
                            if ht_next is not None and do_g1:
                                for q in range(2):
                                    g1_group(xk_next, ht_next, m + 1,
                                             g1_per_g2 * j + q)
                            if do_g2:
                                g2_group(ht, obig, j)
                            if ht_next is not None and do_g1:
                                for q in range(2, g1_per_g2):
                                    g1_group(xk_next, ht_next, m + 1,
                                             g1_per_g2 * j + q)
                            continue
                        if g2_first and do_g2:
                            g2_group(ht, obig, j)
                        if ht_next is not None and do_g1:
                            g1_units(xk_next, ht_next, m + 1, j)
                        if do_g2 and not g2_first:
                            g2_group(ht, obig, j)
                    if do_g2 and "outdma" not in skip and "evict" not in skip:
                        nc.sync.dma_start(
                            out[m * T:(m + 1) * T, :].rearrange(
                                "(j p) h -> p j h", p=P),
                            obig[:])
                    xk, xk_next, ht = xk_next, xk_after, ht_next

    nc.compile()
    return nc


def make_in_maps(inputs_np, mm_mode=_MM_MODE):
    import ml_dtypes

    inputs = np.asarray(inputs_np["inputs"], dtype=np.float32)
    w1 = np.asarray(inputs_np["w1"], dtype=np.float32)
    b1 = np.asarray(inputs_np["b1"], dtype=np.float32)
    w2 = np.asarray(inputs_np["w2"], dtype=np.float32)
    b2 = np.asarray(inputs_np["b2"], dtype=np.float32)

    B, E, C, H = inputs.shape
    tok = B * C
    # [B, E, C, H] -> per-expert TRANSPOSED token matrix [E, H, B*C]
    xT = np.ascontiguousarray(
        inputs.transpose(1, 3, 0, 2).reshape(E, H, tok))
    if mm_mode == "bf16":
        cast = lambda a: np.ascontiguousarray(a).astype(ml_dtypes.bfloat16)
    else:
        cast = lambda a: np.ascontiguousarray(a)

    return [
        {
            "xT": cast(xT[e]),
            "w1": cast(w1[e]),
            "b1": np.ascontiguousarray(b1[e]),
            "w2": cast(w2[e]),
            "b2": np.ascontiguousarray(b2[e]),
        }
        for e in range(E)
    ]


def kernel(inputs, w1, b1, w2, b2):
    from concourse.bass_utils import run_bass_kernel_spmd

    B, E, C, H = inputs.shape

    if "nc" not in _cache:
        _cache["nc"] = build_nc()
    nc = _cache["nc"]

    in_maps = make_in_maps(
        {"inputs": inputs, "w1": w1, "b1": b1, "w2": w2, "b2": b2})
    res = run_bass_kernel_spmd(nc, in_maps, core_ids=list(range(E)))
    o = np.stack([res.results[e]["out"] for e in range(E)])  # [E, tok, H]
    return np.ascontiguousarray(
        o.reshape(E, B, C, H).transpose(1, 0, 2, 3))


# revision 14
# speedup vs baseline: 1.0948x; 1.0311x over previous
"""Expert-parallel MoE FFN kernel for Trainium2 (8 NeuronCores), v2.

Problem: inputs [B=2, E=8, C=8192, H=512], per-expert FFN
    h   = gelu_tanh(x_e @ w1_e + b1_e)        (w1: [E, H, F=2048])
    out = h @ w2_e + b2_e                     (w2: [E, F, H])

Sharding: expert-parallel - core e owns expert e's tokens [B*C, H] and
weights; no cross-core communication.

v2 changes vs v1 (each validated by HW-loop slope timing):
  - x is transposed on the HOST (input marshaling inside kernel()), so
    the device kernel has ZERO PE transposes (v1 spent ~275ns per
    128x128 transpose-mode op).
  - bf16 matmuls (host casts x/w1/w2).  Measured: bf16 streams at the
    same 1 col/cycle as f32r on TRN2 (N=512 MM ~213-220ns), so this
    buys DMA/SBUF headroom, not PE time.  rel_err ~3.4e-3 (gate 2e-2).
  - GEMM2(m) and GEMM1(m+1) interleaved at chain granularity (1 G2
    chain then 4 G1 chains per token-block) so gelu evictions never
    gate the PE; PSUM pools 5+3 banks; deep SBUF pools (4,3,4).
  - Measured budget per core: 4096 MMs x ~213ns stream = 873us floor;
    +~130us intrinsic accumulation-chain restart cost (G1 chains are
    only 4 MMs: K=H=512), +~50us eviction coupling -> ~1.05-1.08ms.

Per-core dataflow:
    GEMM1: ph[f-chunk, tok] += w1[hk, f-chunk].T @ xT[hk, tok-tile]
    gelu on ACT during PSUM->SBUF eviction (bias = b1 per-partition);
    odd groups go via DVE copy + ACT-from-SBUF to avoid back-to-back
    ACT PSUM reads (collapses PE throughput; measured in v1).
    GEMM2: out[tok-blk, :] += hT[fk, tok-blk].T @ w2[fk, :]
    b2 add on DVE during PSUM->SBUF eviction.
"""

import contextlib

import numpy as np

_B, _E, _C, _H, _F = 2, 8, 8192, 512, 2048
_TOK = _B * _C  # 16384 tokens per expert
_P = 128
_T = 512  # tokens per macro tile

_MM_MODE = "bf16"  # "bf16" | "f32r"

_cache = {}


def build_nc(tok=_TOK, act_name="Gelu_apprx_tanh", n_devices=_E,
             mm_mode=_MM_MODE, loop_reps=1, skip=(), psum_cfg=(5, 3),
             g1_per_g2=4, evict_mode="alt", T=_T, bufs_cfg=(4, 3, 4),
             g2_first="block"):
    import concourse.mybir as mybir
    import concourse.tile as tile
    from concourse import bacc

    H, F, P = _H, _F, _P
    HK, FM = H // P, F // P  # 4 contraction chunks, 16 F groups
    FK = FM
    wide = evict_mode == "wide"  # paired G1 chains + single wide gelu
    if wide:
        assert T == 1024
    NJ = T // P  # token sub-blocks per macro tile
    NM = tok // T  # macro tiles
    g1_per_g2 = FM // NJ if not wide else None
    f32 = mybir.dt.float32
    mmdt = mybir.dt.bfloat16 if mm_mode == "bf16" else mybir.dt.float32r
    act = getattr(mybir.ActivationFunctionType, act_name)

    nc = bacc.Bacc("TRN2", debug=False, target_bir_lowering=False,
                   num_devices=n_devices)
    xT = nc.dram_tensor("xT", [H, tok], mmdt, kind="ExternalInput").ap()
    w1 = nc.dram_tensor("w1", [H, F], mmdt, kind="ExternalInput").ap()
    b1 = nc.dram_tensor("b1", [F], f32, kind="ExternalInput").ap()
    w2 = nc.dram_tensor("w2", [F, H], mmdt, kind="ExternalInput").ap()
    b2 = nc.dram_tensor("b2", [H], f32, kind="ExternalInput").ap()
    out = nc.dram_tensor("out", [tok, H], f32, kind="ExternalOutput").ap()

    with tile.TileContext(nc) as tc:
        with (
            tc.tile_pool(name="const", bufs=1) as const,
            tc.tile_pool(name="xin", bufs=bufs_cfg[0]) as xin_pool,
            tc.tile_pool(name="ht", bufs=bufs_cfg[1]) as ht_pool,
            tc.tile_pool(name="obuf", bufs=2) as o_pool,
            tc.tile_pool(name="scr", bufs=bufs_cfg[2]) as scratch,
            tc.tile_pool(name="ps1", bufs=psum_cfg[0], space="PSUM") as ps1,
            tc.tile_pool(name="ps2", bufs=psum_cfg[1], space="PSUM") as ps2,
        ):
            # --- weights / constants, resident in SBUF for the whole kernel
            # weights DMA'd in quarters so the first G1 chains only wait
            # for the first 1MB, overlapping the rest with compute (only
            # matters for the single-dispatch prologue, not the For_i
            # steady state)
            w1_sb = const.tile([P, HK, F], mmdt)
            w2_sb = const.tile([P, FK, H], mmdt)
            for q in range(4):
                fs = q * (F // 4)
                nc.sync.dma_start(
                    w1_sb[:, :, fs:fs + F // 4],
                    w1[:, fs:fs + F // 4].rearrange(
                        "(hk p) f -> p hk f", p=P))
            for q in range(4):
                ks = q * (FK // 4)
                nc.sync.dma_start(
                    w2_sb[:, ks:ks + FK // 4, :],
                    w2[ks * P:(ks + FK // 4) * P, :].rearrange(
                        "(fk p) h -> p fk h", p=P))
            b1_sb = const.tile([P, FM], f32)
            nc.sync.dma_start(b1_sb[:], b1.rearrange("(fm p) -> p fm", p=P))
            b2_row = const.tile([1, H], f32)
            nc.sync.dma_start(b2_row[:], b2[None, :])
            ones = const.tile([1, P], f32)
            nc.any.memset(ones[:], 1.0)
            # broadcast b2 across all 128 partitions via a K=1 matmul
            b2_bc = const.tile([P, H], f32)
            ps_b2 = ps2.tile([P, H], f32, tag="po")
            nc.tensor.matmul(ps_b2[:], ones[:], b2_row[:], start=True,
                             stop=True)
            nc.vector.tensor_copy(b2_bc[:], ps_b2[:])
            static_ht = None
            if "evict" in skip or "gemm1" in skip:  # timing-only variants
                static_ht = const.tile([P, FM, T], mmdt, name="static_ht")
                nc.any.memset(static_ht.rearrange("p fm t -> p (fm t)"), 0.0)

            def load_x(m):
                xk = xin_pool.tile([P, HK, T], mmdt, name=f"xk{m}", tag="xk")
                nc.sync.dma_start(
                    xk[:],
                    xT[:, m * T:(m + 1) * T].rearrange(
                        "(hk p) t -> p hk t", p=P))
                return xk

            def g1_group(xk, ht, m, fm):
                """One GEMM1 accumulation group (4 matmuls) + gelu evict.

                Evictions alternate engines: even fm goes ACT gelu
                straight from PSUM; odd fm is DVE-copied to SBUF scratch
                first and gelu'd from there (back-to-back ACT PSUM reads
                collapse PE throughput; measured in v1).
                """
                ph = ps1.tile([P, T], f32, name=f"ph{m}_{fm}", tag="ph")
                for hk in range(HK):
                    nc.tensor.matmul(
                        ph[:],
                        w1_sb[:, hk, fm * P:(fm + 1) * P],
                        xk[:, hk, :],
                        start=(hk == 0),
                        stop=(hk == HK - 1),
                    )
                if "evict" in skip:
                    return
                direct = (evict_mode == "act" or
                          (evict_mode == "alt" and fm % 2 == 0))
                if direct:
                    nc.scalar.activation(
                        ht[:, fm, :], ph[:], act, bias=b1_sb[:, fm:fm + 1])
                else:
                    sc = scratch.tile([P, T], f32, name=f"sc{m}_{fm}",
                                      tag="sc")
                    nc.vector.tensor_copy(sc[:], ph[:])
                    nc.scalar.activation(
                        ht[:, fm, :], sc[:], act, bias=b1_sb[:, fm:fm + 1])

            def g1_duo(xk, ht, m, fm):
                """Two G1 chains (fm, fm+1) with their matmuls interleaved
                A0 B0 A1 B1 ... across two PSUM banks.  Accumulation state
                is per-bank (has_written), so two open groups may legally
                interleave; if the chain-restart cost is a start-after-stop
                pipeline break, the sibling chain's streaming hides it."""
                pha = ps1.tile([P, T], f32, name=f"ph{m}_{fm}", tag="ph")
                phb = ps1.tile([P, T], f32, name=f"ph{m}_{fm + 1}", tag="ph")
                for hk in range(HK):
                    for ph, f in ((pha, fm), (phb, fm + 1)):
                        nc.tensor.matmul(
                            ph[:],
                            w1_sb[:, hk, f * P:(f + 1) * P],
                            xk[:, hk, :],
                            start=(hk == 0),
                            stop=(hk == HK - 1),
                            skip_group_check=True,
                        )
                if "evict" in skip:
                    return
                nc.scalar.activation(
                    ht[:, fm, :], pha[:], act, bias=b1_sb[:, fm:fm + 1])
                sc = scratch.tile([P, T], f32, name=f"sc{m}_{fm}", tag="sc")
                nc.vector.tensor_copy(sc[:], phb[:])
                nc.scalar.activation(
                    ht[:, fm + 1, :], sc[:], act,
                    bias=b1_sb[:, fm + 1:fm + 2])

            def g1_quad(xk, ht, m, fm):
                """Four G1 chains (fm..fm+3) interleaved at the matmul
                level across four PSUM banks."""
                phs = [ps1.tile([P, T], f32, name=f"ph{m}_{fm + q}",
                                tag="ph") for q in range(4)]
                for hk in range(HK):
                    for q in range(4):
                        nc.tensor.matmul(
                            phs[q][:],
                            w1_sb[:, hk, (fm + q) * P:(fm + q + 1) * P],
                            xk[:, hk, :],
                            start=(hk == 0),
                            stop=(hk == HK - 1),
                            skip_group_check=True,
                        )
                if "evict" in skip:
                    return
                for q in range(4):
                    f = fm + q
                    if q % 2 == 0:
                        nc.scalar.activation(
                            ht[:, f, :], phs[q][:], act,
                            bias=b1_sb[:, f:f + 1])
                    else:
                        sc = scratch.tile([P, T], f32, name=f"sc{m}_{f}",
                                          tag="sc")
                        nc.vector.tensor_copy(sc[:], phs[q][:])
                        nc.scalar.activation(
                            ht[:, f, :], sc[:], act, bias=b1_sb[:, f:f + 1])

            def g1_pair(xk, ht, m, fm):
                """Two GEMM1 chains for the SAME fm over adjacent 512-token
                halves -> one 2-bank PSUM tile, evicted by a single wide
                [128,1024] ACT gelu.  Both halves share fm's per-partition
                bias, so this is exact for any b1."""
                ph = ps1.tile([P, 2, T // 2], f32, name=f"php{m}_{fm}",
                              tag="ph")
                for half in range(2):
                    for hk in range(HK):
                        nc.tensor.matmul(
                            ph[:, half, :],
                            w1_sb[:, hk, fm * P:(fm + 1) * P],
                            xk[:, hk, half * (T // 2):(half + 1) * (T // 2)],
                            start=(hk == 0),
                            stop=(hk == HK - 1),
                        )
                if "evict" in skip:
                    return
                nc.scalar.activation(
                    ht[:, fm, :], ph.rearrange("p h t -> p (h t)"), act,
                    bias=b1_sb[:, fm:fm + 1])

            def g2_group(ht, obig, j):
                """One GEMM2 accumulation group (16 matmuls) + b2 evict."""
                po = ps2.tile([P, H], f32, tag="po", name="po")
                for fk in range(FK):
                    nc.tensor.matmul(
                        po[:],
                        ht[:, fk, j * P:(j + 1) * P],
                        w2_sb[:, fk, :],
                        start=(fk == 0),
                        stop=(fk == FK - 1),
                    )
                if "evict" not in skip:
                    nc.vector.tensor_add(obig[:, j, :], po[:], b2_bc[:])

            def new_ht(m):
                if static_ht is not None:
                    return static_ht
                return ht_pool.tile([P, FM, T], mmdt, name=f"ht{m}", tag="ht")

            do_g1 = "gemm1" not in skip
            do_g2 = "gemm2" not in skip
            # loop_reps > 1 wraps the body in a hardware loop - used only
            # by the timing harness to amortize per-dispatch overhead.
            reps_ctx = (tc.For_i(0, loop_reps, 1) if loop_reps > 1
                        else contextlib.nullcontext())
            with reps_ctx:
                # software pipeline, one macro of skew: G1 fills ht(m+1)
                # while G2 consumes ht(m).
                def g1_units(xk, ht, m, j):
                    """G1 emission units for token-slot j of macro m."""
                    if wide:  # 2 pairs per j-slot (16 pairs over 8 slots)
                        for q in range(2):
                            g1_pair(xk, ht, m, 2 * j + q)
                    elif evict_mode == "duo":
                        for q in range(g1_per_g2 // 2):
                            g1_duo(xk, ht, m, g1_per_g2 * j + 2 * q)
                    elif evict_mode == "quad":
                        g1_quad(xk, ht, m, g1_per_g2 * j)
                    else:
                        for q in range(g1_per_g2):
                            g1_group(xk, ht, m, g1_per_g2 * j + q)

                xk = load_x(0)
                xk_next = load_x(1)
                ht = new_ht(0)
                if do_g1:
                    for j in range(NJ):
                        g1_units(xk, ht, 0, j)
                for m in range(NM):
                    if m + 2 < NM:
                        xk_after = load_x(m + 2)
                    else:
                        xk_after = None
                    ht_next = new_ht(m + 1) if (m + 1 < NM) else None
                    obig = (None if "evict" in skip else
                            o_pool.tile([P, NJ, H], f32, name=f"ob{m}",
                                        tag="ob"))
                    if g2_first == "block":
                        # all G2 chains consecutively, then all G1: only 2
                        # G1<->G2 pool transitions per macro instead of 8
                        # (each transition costs a chain-restart, ~180-250ns)
                        if do_g2:
                            for j in range(NJ):
                                g2_group(ht, obig, j)
                        if ht_next is not None and do_g1:
                            for j in range(NJ):
                                g1_units(xk_next, ht_next, m + 1, j)
                        if do_g2 and "outdma" not in skip and "evict" not in skip:
                            nc.sync.dma_start(
                                out[m * T:(m + 1) * T, :].rearrange(
                                    "(j p) h -> p j h", p=P),
                                obig[:])
                        xk, xk_next, ht = xk_next, xk_after, ht_next
                        continue
                    for j in range(NJ):
                        if g2_first == "mid" and not wide:
                            if ht_next is not None and do_g1:
                                for q in range(2):
                                    g1_group(xk_next, ht_next, m + 1,
                                             g1_per_g2 * j + q)
                            if do_g2:
                                g2_group(ht, obig, j)
                            if ht_next is not None and do_g1:
                                for q in range(2, g1_per_g2):
                                    g1_group(xk_next, ht_next, m + 1,
                                             g1_per_g2 * j + q)
                            continue
                        if g2_first and do_g2:
                            g2_group(ht, obig, j)
                        if ht_next is not None and do_g1:
                            g1_units(xk_next, ht_next, m + 1, j)
                        if do_g2 and not g2_first:
                            g2_group(ht, obig, j)
                    if do_g2 and "outdma" not in skip and "evict" not in skip:
                        nc.sync.dma_start(
                            out[m * T:(m + 1) * T, :].rearrange(
                                "(j p) h -> p j h", p=P),
                            obig[:])
                    xk, xk_next, ht = xk_next, xk_after, ht_next

    nc.compile()
    return nc


def make_in_maps(inputs_np, mm_mode=_MM_MODE):
    import ml_dtypes

    inputs = np.asarray(inputs_np["inputs"], dtype=np.float32)
    w1 = np.asarray(inputs_np["w1"], dtype=np.float32)
    b1 = np.asarray(inputs_np["b1"], dtype=np.float32)
    w2 = np.asarray(inputs_np["w2"], dtype=np.float32)
    b2 = np.asarray(inputs_np["b2"], dtype=np.float32)

    B, E, C, H = inputs.shape
    tok = B * C
    # [B, E, C, H] -> per-expert TRANSPOSED token matrix [E, H, B*C]
    xT = np.ascontiguousarray(
        inputs.transpose(1, 3, 0, 2).reshape(E, H, tok))
    if mm_mode == "bf16":
        cast = lambda a: np.ascontiguousarray(a).astype(ml_dtypes.bfloat16)
    else:
        cast = lambda a: np.ascontiguousarray(a)

    return [
        {
            "xT": cast(xT[e]),
            "w1": cast(w1[e]),
            "b1": np.ascontiguousarray(b1[e]),
            "w2": cast(w2[e]),
            "b2": np.ascontiguousarray(b2[e]),
        }
        for e in range(E)
    ]


def kernel(inputs, w1, b1, w2, b2):
    from concourse.bass_utils import run_bass_kernel_spmd

    B, E, C, H = inputs.shape

    if "nc" not in _cache:
        _cache["nc"] = build_nc()
    nc = _cache["nc"]

    in_maps = make_in_maps(
        {"inputs": inputs, "w1": w1, "b1": b1, "w2": w2, "b2": b2})
    res = run_bass_kernel_spmd(nc, in_maps, core_ids=list(range(E)))
    o = np.stack([res.results[e]["out"] for e in range(E)])  # [E, tok, H]
    return np.ascontiguousarray(
        o.reshape(E, B, C, H).transpose(1, 0, 2, 3))
